# revision 1
# baseline (speedup 1.0000x reference)
"""DeeperHNN hypergraph message passing kernel for 8 Trainium2 NeuronCores.

Strategy (sharding_hint): nodes (and incidence entries, partitioned by vertex)
are sharded across 8 cores; hyperedge aggregates are computed as per-core
partials and AllReduced (replicated) since E << N; weight matrices replicated.

Per conv layer, per core:
  T = h @ thetaW[i] + thetaB[i]                     (row-major, DRAM)
  Phase A: partial_Ye[e] = sum_{local v in e} de_inv[e] * T[v]
           = gather T rows by vidx (dma_gather) -> one-hot segment matmul on PE
           -> contiguous 128-edge block writes
  AllReduce(partial_Ye) -> Ye (replicated)
  Phase B: conv[v] = relu(dv_inv[v] * sum_{e: v in e} Ye[e])
           = gather Ye rows by eidx -> one-hot segment matmul -> 128-node blocks
  h' = h + conv (residual, layers 1-3); tail: t = relu(LN(h')), hT = t^T for the
  next layer's matmul.

All segment structure (slot counts, token index tables, one-hot row ids,
de_inv weights) is precomputed on host from vidx/eidx and shipped as int16/f32
metadata tables resident in SBUF.
"""

import math

import numpy as np

import concourse.bacc as bacc
import concourse.bass as bass
import concourse.mybir as mybir
import concourse.tile as tile
from concourse.bass_utils import run_bass_kernel_spmd
from concourse.masks import make_identity

import ml_dtypes

P = 128
BF16_NP = ml_dtypes.bfloat16
USE_BF16 = True
F32 = mybir.dt.float32
BF16 = mybir.dt.bfloat16
I16 = mybir.dt.int16
I32 = mybir.dt.int32
AF = mybir.ActivationFunctionType
ALU = mybir.AluOpType


def _cdiv(a, b):
    return (a + b - 1) // b


# ----------------------------------------------------------------------------
# Host-side preprocessing: build per-core token tables from vidx/eidx.
# ----------------------------------------------------------------------------
class Prep:
    pass


def host_prep(vidx, eidx, N, E, C, gmax_a=24, gmax_b=20):
    """Build the static segment/gather structure shared by the SPMD program.

    Tokens for phase A (v->e): per core, entries sorted by eidx, grouped into
    157 blocks of 128 edges; each block padded to a whole number of 128-token
    slots (count = max over cores, so the program is identical on all cores).
    Phase B (e->v) is the same with the roles of (node block, eidx) swapped.
    """
    p = Prep()
    NP = N // C
    NBE = _cdiv(E, P)
    NBV = _cdiv(NP, P)
    NPAD = NBV * P
    EPAD = NBE * P
    p.N, p.E, p.C, p.NP, p.NBE, p.NBV, p.NPAD, p.EPAD = N, E, C, NP, NBE, NBV, NPAD, EPAD

    vidx = np.asarray(vidx).astype(np.int64)
    eidx = np.asarray(eidx).astype(np.int64)
    de = np.bincount(eidx, minlength=E).astype(np.float64)
    dv = np.bincount(vidx, minlength=N).astype(np.float64)
    de_inv = (1.0 / np.maximum(de, 1.0)).astype(np.float32)
    dv_inv = (1.0 / np.maximum(dv, 1.0)).astype(np.float32)
    core = vidx // NP

    # ---- phase A ----
    A_ev, A_lv = [], []
    cntA = np.zeros((C, NBE), np.int64)
    for c in range(C):
        m = core == c
        ev = eidx[m]
        lv = vidx[m] - c * NP
        o = np.argsort(ev, kind="stable")
        ev, lv = ev[o], lv[o]
        cntA[c] = np.bincount(ev // P, minlength=NBE)
        A_ev.append(ev)
        A_lv.append(lv)
    slotsA = np.maximum(1, _cdiv(cntA.max(0), P)).astype(np.int64)
    SA = int(slotsA.sum())
    TA = SA * P
    offA = np.zeros(NBE + 1, np.int64)
    np.cumsum(slotsA * P, out=offA[1:])

    idxA = np.full((C, TA), NP, np.int16)  # dummy -> zero row of T
    rA = np.zeros((C, TA), np.float32)
    for c in range(C):
        ev, lv = A_ev[c], A_lv[c]
        blk = ev // P
        starts = np.searchsorted(ev, np.arange(NBE) * P)
        tok = offA[blk] + (np.arange(len(ev)) - starts[blk])
        idxA[c, tok] = lv
        rA[c, tok] = ev - blk * P

    # ---- phase B ----
    B_ee, B_lv = [], []
    cntB = np.zeros((C, NBV), np.int64)
    for c in range(C):
        m = core == c
        lv = vidx[m] - c * NP
        ee = eidx[m]
        o = np.argsort(lv, kind="stable")
        lv, ee = lv[o], ee[o]
        cntB[c] = np.bincount(lv // P, minlength=NBV)
        B_ee.append(ee)
        B_lv.append(lv)
    slotsB = np.maximum(1, _cdiv(cntB.max(0), P)).astype(np.int64)
    SB = int(slotsB.sum())
    TB = SB * P
    offB = np.zeros(NBV + 1, np.int64)
    np.cumsum(slotsB * P, out=offB[1:])

    idxB = np.full((C, TB), E, np.int16)  # dummy -> zeroed row E of Ye
    rB = np.zeros((C, TB), np.float32)
    for c in range(C):
        ee, lv = B_ee[c], B_lv[c]
        blk = lv // P
        starts = np.searchsorted(lv, np.arange(NBV) * P)
        tok = offB[blk] + (np.arange(len(lv)) - starts[blk])
        idxB[c, tok] = ee
        rB[c, tok] = lv - blk * P

    # per-node dv_inv columns [C, 128, NBV]
    dvc = np.zeros((C, P, NBV), np.float32)
    for c in range(C):
        ids = c * NP + np.arange(NPAD)
        vals = np.where(ids < (c + 1) * NP, dv_inv[np.minimum(ids, N - 1)], 0.0)
        dvc[c] = vals.reshape(NBV, P).T

    # wrapped layouts for the device
    p.slotsA, p.slotsB, p.SA, p.SB, p.TA, p.TB = slotsA, slotsB, SA, SB, TA, TB
    p.offA, p.offB = offA, offB
    p.idxA_w = np.ascontiguousarray(np.tile(idxA.reshape(C, TA // 16, 16).transpose(0, 2, 1), (1, 8, 1)))
    rdt = BF16_NP if USE_BF16 else np.float32
    p.rA_m = np.ascontiguousarray(rA.reshape(C, SA, P).transpose(0, 2, 1)).astype(rdt)
    p.idxB_w = np.ascontiguousarray(np.tile(idxB.reshape(C, TB // 16, 16).transpose(0, 2, 1), (1, 8, 1)))
    p.rB_m = np.ascontiguousarray(rB.reshape(C, SB, P).transpose(0, 2, 1)).astype(rdt)
    # de_inv per edge-block column [128, NBE] (same on all cores)
    dec = np.zeros(EPAD, np.float32)
    dec[:E] = de_inv
    p.dec = dec.reshape(NBE, P).T.copy()
    p.dvc = dvc
    p.MAXSLOT = int(max(slotsA.max(), slotsB.max()))

    # gather groups: consecutive blocks, total slots <= gmax
    def make_groups(slots, gmax):
        groups = []  # (block0, nblocks, slot0, gslots)
        b = 0
        nb = len(slots)
        while b < nb:
            s0 = int(slots[:b].sum())
            g = 0
            n = 0
            while b + n < nb and g + slots[b + n] <= gmax:
                g += int(slots[b + n])
                n += 1
            assert n > 0, "single block exceeds gmax"
            groups.append((b, n, s0, g))
            b += n
        return groups

    p.gmax_a, p.gmax_b = gmax_a, gmax_b
    p.groupsA = make_groups(slotsA, gmax_a)
    p.groupsB = make_groups(slotsB, gmax_b)
    return p


# ----------------------------------------------------------------------------
# Device program
# ----------------------------------------------------------------------------
def build_program(p, IN_DIM, H, OUT, L, enable_asserts=False, stage=99):
    C, NP, NBE, NBV, NPAD, EPAD = p.C, p.NP, p.NBE, p.NBV, p.NPAD, p.EPAD
    KI = IN_DIM // P  # input-dim K tiles (3)
    KH = H // P  # hidden K tiles (2)
    assert IN_DIM % P == 0 and H % P == 0

    nc = bacc.Bacc(
        "TRN2",
        target_bir_lowering=False,
        debug=False,
        enable_asserts=enable_asserts,
        num_devices=C,
        num_swdge_queues=4,
    )

    # ---- I/O ----
    xT_d = nc.dram_tensor("xT", [IN_DIM, NPAD], F32, kind="ExternalInput")
    encW_d = nc.dram_tensor("encW", [IN_DIM, H], F32, kind="ExternalInput")
    encB_d = nc.dram_tensor("encB", [H], F32, kind="ExternalInput")
    thW_d = nc.dram_tensor("thW", [L, H, H], F32, kind="ExternalInput")
    thB_d = nc.dram_tensor("thB", [L, H], F32, kind="ExternalInput")
    lnG_d = nc.dram_tensor("lnG", [L, H], F32, kind="ExternalInput")
    lnB_d = nc.dram_tensor("lnB", [L, H], F32, kind="ExternalInput")
    linW_d = nc.dram_tensor("linW", [H, OUT], F32, kind="ExternalInput")
    linB_d = nc.dram_tensor("linB", [OUT], F32, kind="ExternalInput")
    idxA_d = nc.dram_tensor("idxA", [P, p.TA // 16], I16, kind="ExternalInput")
    GDT = BF16 if USE_BF16 else F32
    rA_d = nc.dram_tensor("rA", [P, p.SA], GDT, kind="ExternalInput")
    idxB_d = nc.dram_tensor("idxB", [P, p.TB // 16], I16, kind="ExternalInput")
    rB_d = nc.dram_tensor("rB", [P, p.SB], GDT, kind="ExternalInput")
    dv_d = nc.dram_tensor("dvc", [P, NBV], F32, kind="ExternalInput")
    dec_d = nc.dram_tensor("dec", [P, NBE], F32, kind="ExternalInput")
    out_d = nc.dram_tensor("out", [NP, OUT], F32, kind="ExternalOutput")

    # ---- internals ----
    TEXT = P if NP == NPAD else 0  # room for the dummy row when NP % 128 == 0
    EEXT = P if p.E == EPAD else 0
    T_d = nc.dram_tensor("T_t", [NPAD + TEXT, H], GDT)  # row NP is the zero dummy
    YeP_d = nc.dram_tensor("YeP", [EPAD, H], GDT)  # rows E.. end up zero
    YeF_d = nc.dram_tensor(
        "YeF", [EPAD + EEXT, H], GDT,
        addr_space="Shared" if C > 4 else "Local",
    )
    h_d = nc.dram_tensor("h_t", [NPAD, H], F32)
    hT_d = nc.dram_tensor("hT", [H, NPAD], F32)

    last_rows = NP - (NBV - 1) * P  # valid rows in the final node block

    from contextlib import ExitStack
    with tile.TileContext(nc) as tc, ExitStack() as es:
        const = es.enter_context(tc.tile_pool(name="const", bufs=1))
        meta = es.enter_context(tc.tile_pool(name="meta", bufs=1))
        gpool = es.enter_context(tc.tile_pool(name="gpool", bufs=2))
        spool = es.enter_context(tc.tile_pool(name="spool", bufs=3))
        wrk = es.enter_context(tc.tile_pool(name="wrk", bufs=3))
        stat = es.enter_context(tc.tile_pool(name="stat", bufs=4))
        opool = es.enter_context(tc.tile_pool(name="opool", bufs=3))
        psA = es.enter_context(tc.tile_pool(name="psA", bufs=3, space="PSUM"))
        psT = es.enter_context(tc.tile_pool(name="psT", bufs=2, space="PSUM"))
        psE = es.enter_context(tc.tile_pool(name="psE", bufs=2, space="PSUM"))

        # ---- constants ----
        iota_i = const.tile([P, p.MAXSLOT, P], I32)
        nc.gpsimd.iota(iota_i[:, :, :], pattern=[[0, p.MAXSLOT], [1, P]], base=0,
                       channel_multiplier=0)
        iota_f = const.tile([P, p.MAXSLOT, P], GDT)
        nc.vector.tensor_copy(iota_f[:, :, :], iota_i[:, :, :])
        ident = const.tile([P, P], F32)
        make_identity(nc, ident[:, :])
        ones1 = const.tile([1, P], F32)
        nc.vector.memset(ones1[:, :], 1.0)
        epsc = const.tile([P, 1], F32)
        nc.vector.memset(epsc[:, :], 1e-5)
        zrow = const.tile([1, H], F32)
        nc.vector.memset(zrow[:, :], 0.0)

        # weights
        encW_t = []
        for k in range(KI):
            row = []
            for m in range(KH):
                t = const.tile([P, P], F32, tag=f"encW{k}{m}")
                nc.sync.dma_start(t[:, :], encW_d[k * P:(k + 1) * P, m * P:(m + 1) * P])
                row.append(t)
            encW_t.append(row)
        encB_c = []
        for m in range(KH):
            t = const.tile([P, 1], F32, tag=f"encB{m}")
            nc.sync.dma_start(t[:, :], encB_d[m * P:(m + 1) * P, None])
            encB_c.append(t)
        thW_t = []
        for i in range(L):
            row = []
            for k in range(KH):
                t = const.tile([P, H], F32, tag=f"thW{i}{k}")
                nc.sync.dma_start(t[:, :], thW_d[i, k * P:(k + 1) * P, :])
                row.append(t)
            thW_t.append(row)
        thB_t = []
        for i in range(L):
            t = const.tile([1, H], F32, tag=f"thB{i}")
            nc.sync.dma_start(t[:, :], thB_d[i:i + 1, :])
            thB_t.append(t)
        linW_t = []
        for k in range(KH):
            t = const.tile([P, OUT], F32, tag=f"linW{k}")
            nc.sync.dma_start(t[:, :], linW_d[k * P:(k + 1) * P, :])
            linW_t.append(t)
        linB_t = const.tile([1, OUT], F32)
        nc.sync.dma_start(linB_t[:, :], linB_d[None, :])
        lnG_t, lnB_t = [], []
        for i in range(L):
            g = const.tile([P, H], F32, tag=f"lnG{i}")
            b = const.tile([P, H], F32, tag=f"lnB{i}")
            nc.sync.dma_start(g[:, :], lnG_d[i:i + 1, :].partition_broadcast(P).squeeze(1))
            nc.sync.dma_start(b[:, :], lnB_d[i:i + 1, :].partition_broadcast(P).squeeze(1))
            lnG_t.append(g)
            lnB_t.append(b)

        # metadata
        idxA_t = meta.tile([P, p.TA // 16], I16)
        nc.sync.dma_start(idxA_t[:, :], idxA_d[:, :])
        rA_t = meta.tile([P, p.SA], GDT)
        nc.sync.dma_start(rA_t[:, :], rA_d[:, :])
        dec_t = meta.tile([P, NBE], F32)
        nc.sync.dma_start(dec_t[:, :], dec_d[:, :])
        idxB_t = meta.tile([P, p.TB // 16], I16)
        nc.sync.dma_start(idxB_t[:, :], idxB_d[:, :])
        rB_t = meta.tile([P, p.SB], GDT)
        nc.sync.dma_start(rB_t[:, :], rB_d[:, :])
        dv_t = meta.tile([P, NBV], F32)
        nc.sync.dma_start(dv_t[:, :], dv_d[:, :])

        # zero the dummy/pad zones that gathers may read
        zblk = const.tile([P, H], GDT)
        nc.vector.memset(zblk[:, :], 0.0)
        r0 = NP
        while r0 < NPAD + TEXT:
            nr = min(P, NPAD + TEXT - r0)
            nc.sync.dma_start(T_d[r0:r0 + nr, :], zblk[:nr, :])
            r0 += nr
        if EEXT:
            nc.sync.dma_start(YeF_d[EPAD:EPAD + EEXT, :], zblk[:EEXT, :])

        # ------------------------------------------------------------------
        # Encoder: hT = (x @ encW + encB)^T, computed directly feature-major.
        # ------------------------------------------------------------------
        CW = 512
        for c0 in range(0, NPAD, CW):
            ncols = min(CW, NPAD - c0)
            xc = wrk.tile([P, KI, CW], F32, tag="xc")
            nc.sync.dma_start(
                xc[:, :, :ncols],
                xT_d.ap().rearrange("(k q) n -> q k n", q=P)[:, :, c0:c0 + ncols],
            )
            for m in range(KH):
                ps = psE.tile([P, CW], F32, tag="psE")
                for k in range(KI):
                    nc.tensor.matmul(ps[:, :ncols], lhsT=encW_t[k][m][:, :],
                                     rhs=xc[:, k, :ncols],
                                     start=(k == 0), stop=(k == KI - 1))
                ob = opool.tile([P, CW], F32, tag="encout")
                nc.scalar.activation(ob[:, :ncols], ps[:, :ncols], AF.Identity,
                                     bias=encB_c[m][:, :], scale=1.0)
                nc.scalar.dma_start(hT_d[m * P:(m + 1) * P, c0:c0 + ncols], ob[:, :ncols])

        hT_view = hT_d.ap().rearrange("(k q) n -> q k n", q=P)

        # ------------------------------------------------------------------
        # Conv layers
        # ------------------------------------------------------------------
        for li in range(L if stage >= 2 else 0):
            # ---- T = h @ thetaW[li] + thetaB[li] (row-major) ----
            for rb in range(NBV):
                rows = last_rows if rb == NBV - 1 else P
                hTl = wrk.tile([P, KH, P], F32, tag="hTl")
                nc.sync.dma_start(hTl[:, :, :], hT_view[:, :, rb * P:rb * P + P])
                ps = psA.tile([P, H], F32, tag="ps256")
                for k in range(KH):
                    nc.tensor.matmul(ps[:, :], lhsT=hTl[:, k, :], rhs=thW_t[li][k][:, :],
                                     start=(k == 0), stop=False)
                nc.tensor.matmul(ps[:, :], lhsT=ones1[:1, :], rhs=thB_t[li][:1, :],
                                 start=False, stop=True)
                Tb = opool.tile([P, H], GDT, tag="Tout")
                nc.scalar.activation(Tb[:, :], ps[:, :], AF.Copy)
                nc.sync.dma_start(T_d[rb * P:rb * P + rows, :], Tb[:rows, :])

            # ---- Phase A: partial Ye ----
            qn = 0
            for eb in (range(NBE) if stage >= 3 else []):
                sb = int(p.slotsA[eb])
                s0 = int(p.offA[eb]) // P
                G = gpool.tile([P, p.MAXSLOT, H], GDT, tag="G")
                g0 = 0
                while g0 < sb:
                    gs = min(8, sb - g0)
                    tok0 = (s0 + g0) * P
                    nc.gpsimd.dma_gather(
                        out_ap=G[:, g0:g0 + gs, :],
                        in_ap=T_d[:, :],
                        idxs_ap=idxA_t[:, tok0 // 16:(tok0 + gs * P) // 16],
                        num_idxs=gs * P,
                        num_idxs_reg=gs * P,
                        elem_size=H,
                        queue_num=qn,
                    )
                    qn = (qn + 1) % 4
                    g0 += gs
                S = spool.tile([P, p.MAXSLOT, P], GDT, tag="SA")
                rb_ap = rA_t[:, s0:s0 + sb].unsqueeze(2).broadcast_to([P, sb, P])
                nc.vector.tensor_tensor(S[:, :sb, :], iota_f[:, :sb, :], rb_ap,
                                        op=ALU.is_equal)
                ps = psA.tile([P, H], F32, tag="ps256")
                for s in range(sb):
                    nc.tensor.matmul(ps[:, :], lhsT=S[:, s, :], rhs=G[:, s, :],
                                     start=(s == 0), stop=(s == sb - 1))
                yeb = opool.tile([P, H], GDT, tag="yeg")
                nc.scalar.activation(yeb[:, :], ps[:, :], AF.Copy,
                                     scale=dec_t[:, eb:eb + 1])
                nc.sync.dma_start(YeP_d[eb * P:(eb + 1) * P, :], yeb[:, :])

            # ---- AllReduce hyperedge partials ----
            if stage < 4:
                continue
            nc.gpsimd.collective_compute(
                "AllReduce",
                ALU.add,
                replica_groups=[list(range(C))],
                ins=[YeP_d.ap()[:EPAD, :]],
                outs=[YeF_d.ap()[:EPAD, :]],
            )

            # ---- Phase B: conv output + residual + LN tail ----
            lnxt = li + 1 if li + 1 < L else 0
            for vb in (range(NBV) if stage >= 5 else []):
                rows = last_rows if vb == NBV - 1 else P
                sb = int(p.slotsB[vb])
                s0 = int(p.offB[vb]) // P
                G = gpool.tile([P, p.MAXSLOT, H], GDT, tag="G")
                g0 = 0
                while g0 < sb:
                    gs = min(8, sb - g0)
                    tok0 = (s0 + g0) * P
                    nc.gpsimd.dma_gather(
                        out_ap=G[:, g0:g0 + gs, :],
                        in_ap=YeF_d[:, :],
                        idxs_ap=idxB_t[:, tok0 // 16:(tok0 + gs * P) // 16],
                        num_idxs=gs * P,
                        num_idxs_reg=gs * P,
                        elem_size=H,
                        queue_num=qn,
                    )
                    qn = (qn + 1) % 4
                    g0 += gs
                if True:
                    S = spool.tile([P, p.MAXSLOT, P], GDT, tag="SA")
                    rb_ap = rB_t[:, s0:s0 + sb].unsqueeze(2).broadcast_to([P, sb, P])
                    nc.vector.tensor_tensor(S[:, :sb, :], iota_f[:, :sb, :], rb_ap,
                                            op=ALU.is_equal)
                    ps = psA.tile([P, H], F32, tag="ps256")
                    for s in range(sb):
                        nc.tensor.matmul(ps[:, :], lhsT=S[:, s, :], rhs=G[:, s, :],
                                         start=(s == 0), stop=(s == sb - 1))
                    # relu(dv * x) (== dv * relu(x), dv >= 0)
                    hn = wrk.tile([P, H], F32, tag="hn")
                    nc.scalar.activation(hn[:, :], ps[:, :], AF.Relu,
                                         scale=dv_t[:, vb:vb + 1])
                    if li > 0:
                        hp = wrk.tile([P, H], F32, tag="hp")
                        nc.scalar.dma_start(hp[:rows, :], h_d[vb * P:vb * P + rows, :])
                        nc.vector.tensor_add(hn[:rows, :], hn[:rows, :], hp[:rows, :])
                    nc.scalar.dma_start(h_d[vb * P:vb * P + rows, :], hn[:rows, :])

                    # tail: t = relu(LN_lnxt(hn)); hT = t^T
                    if stage < 6:
                        continue
                    st6 = stat.tile([P, 6], F32, tag="st6")
                    nc.vector.bn_stats(st6[:, :], hn[:, :])
                    mv = stat.tile([P, 2], F32, tag="mv")
                    nc.vector.bn_aggr(mv[:, :], st6[:, :])
                    rstd = stat.tile([P, 1], F32, tag="rstd")
                    nc.scalar.activation(rstd[:, :], mv[:, 1:2], AF.Sqrt,
                                         bias=epsc[:, :], scale=1.0)
                    rinv = stat.tile([P, 1], F32, tag="rinv")
                    nc.vector.reciprocal(rinv[:, :], rstd[:, :])
                    tt = wrk.tile([P, H], F32, tag="tt")
                    nc.vector.tensor_scalar(tt[:, :], hn[:, :], mv[:, 0:1], rinv[:, :],
                                            op0=ALU.subtract, op1=ALU.mult)
                    nc.vector.tensor_tensor(tt[:, :], tt[:, :], lnG_t[lnxt][:, :],
                                            op=ALU.mult)
                    nc.vector.tensor_tensor(tt[:, :], tt[:, :], lnB_t[lnxt][:, :],
                                            op=ALU.add)
                    nc.vector.tensor_scalar_max(tt[:, :], tt[:, :], 0.0)
                    tT = opool.tile([P, KH, P], F32, tag="tT")
                    for m in range(KH):
                        pst = psT.tile([P, P], F32, tag="psT")
                        nc.tensor.transpose(pst[:, :], tt[:, m * P:(m + 1) * P], ident[:, :])
                        nc.scalar.activation(tT[:, m, :], pst[:, :], AF.Copy)
                    nc.scalar.dma_start(
                        hT_view[:, :, vb * P:vb * P + rows],
                        tT[:, :, :rows],
                    )

        # ------------------------------------------------------------------
        # Final: out = t @ linW + linB  (t == relu(LN_0(h)) already in hT)
        # ------------------------------------------------------------------
        for rb in range(NBV):
            rows = last_rows if rb == NBV - 1 else P
            hTl = wrk.tile([P, KH, P], F32, tag="hTl")
            nc.sync.dma_start(hTl[:, :, :], hT_view[:, :, rb * P:rb * P + P])
            ps = psT.tile([P, OUT], F32, tag="psT")
            for k in range(KH):
                nc.tensor.matmul(ps[:, :], lhsT=hTl[:, k, :], rhs=linW_t[k][:, :],
                                 start=(k == 0), stop=False)
            nc.tensor.matmul(ps[:, :], lhsT=ones1[:1, :], rhs=linB_t[:1, :],
                             start=False, stop=True)
            ob = opool.tile([P, OUT], F32, tag="finout")
            nc.scalar.activation(ob[:, :], ps[:, :], AF.Copy)
            nc.sync.dma_start(out_d[rb * P:rb * P + rows, :], ob[:rows, :])

    nc.compile()
    return nc


# ----------------------------------------------------------------------------
# Full pipeline: prep + build + run
# ----------------------------------------------------------------------------
def run_full(x, vidx, eidx, encW, encB, thetaW, thetaB, lnG, lnB, linW, linB,
             N, E, C, trace=False, nc_cache=None, stage=99, **runkw):
    IN_DIM = x.shape[1]
    H = encW.shape[1]
    OUT = linW.shape[1]
    L = thetaW.shape[0]

    p = host_prep(np.asarray(vidx), np.asarray(eidx), N, E, C)
    nc = nc_cache if nc_cache is not None else build_program(p, IN_DIM, H, OUT, L, stage=stage)

    x = np.asarray(x, np.float32)
    NP, NPAD = p.NP, p.NPAD
    in_maps = []
    for c in range(C):
        xs = x[c * NP:(c + 1) * NP]
        xT = np.zeros((IN_DIM, NPAD), np.float32)
        xT[:, :NP] = xs.T
        in_maps.append(dict(
            xT=xT,
            encW=np.asarray(encW, np.float32),
            encB=np.asarray(encB, np.float32),
            thW=np.asarray(thetaW, np.float32),
            thB=np.asarray(thetaB, np.float32),
            lnG=np.asarray(lnG, np.float32),
            lnB=np.asarray(lnB, np.float32),
            linW=np.asarray(linW, np.float32),
            linB=np.asarray(linB, np.float32),
            idxA=p.idxA_w[c],
            rA=p.rA_m[c],
            idxB=p.idxB_w[c],
            rB=p.rB_m[c],
            dvc=p.dvc[c],
            dec=p.dec,
        ))

    res = run_bass_kernel_spmd(nc, in_maps, core_ids=list(range(C)), trace=trace, **runkw)
    out = np.concatenate([res.results[c]["out"] for c in range(C)], axis=0)
    return out, res, nc, p


# hardcoded problem configuration (nn_DeeperHNN_88295937671288)
_N, _E, _NNZ = 100000, 20000, 800000
_C = 8

_nc_cache = None


def kernel(x, vidx, eidx, encW, encB, thetaW, thetaB, lnG, lnB, linW, linB):
    global _nc_cache
    out, res, nc, p = run_full(
        x, vidx, eidx, encW, encB, thetaW, thetaB, lnG, lnB, linW, linB,
        N=_N, E=_E, C=_C, nc_cache=None,
    )
    _nc_cache = nc
    return out.astype(np.float32)



# revision 13
# speedup vs baseline: 1.9802x; 1.9802x over previous
"""DeeperHNN hypergraph message passing kernel for 8 Trainium2 NeuronCores.

Strategy (sharding_hint): nodes (and incidence entries, partitioned by vertex)
are sharded across 8 cores; hyperedge aggregates are computed as per-core
partials and AllReduced (replicated, chunked for overlap); weights replicated.

v2 design vs v1 baseline (9.5ms):
  - fp8(e4m3) T rows for the phase-A gather (error averages over ~40-deg
    edges); fp16 for everything else 16-bit (Ye, weights, hT, h).
  - Unpadded gathers: per-block num_idxs = roundup16(max-core count), tables
    padded with -1 indices (skipped by the DMA -> no HBM transfer); the
    one-hot row-position tables use -1 so padding contributes zero.
  - hT (transposed activations) resident in SBUF across the whole kernel.
  - AllReduce split into 2 edge-chunks so chunk 0 reduces while phase A
    still computes chunk 1.
  - LN affine + relu fused into the transpose's PSUM->SBUF copy (per-feature
    scale/bias become per-partition in transposed space).
  - Residual stream h in DRAM fp16.

Per conv layer, per core:
  T = h @ thetaW[i] + thetaB[i]              (fp16 matmul from SBUF hT)
  Phase A: gather T rows (fp8) by token -> one-hot segment matmul -> YeP
  chunked AllReduce(YeP) -> YeF (fp16, replicated)
  Phase B: gather YeF rows (fp16) -> one-hot segment matmul -> relu(dv*x)
  h' = h + conv; tail: z=(h'-mu)*rinv, transpose, fused relu(g*zT+b) -> hT
"""

import numpy as np

import concourse.bacc as bacc
import concourse.bass as bass
import concourse.mybir as mybir
import concourse.tile as tile
from concourse.bass_utils import run_bass_kernel_spmd
from concourse.masks import make_identity

import ml_dtypes

P = 128
F32 = mybir.dt.float32
F16 = mybir.dt.float16
F8 = mybir.dt.float8e4
I16 = mybir.dt.int16
I32 = mybir.dt.int32
AF = mybir.ActivationFunctionType
ALU = mybir.AluOpType

F16_NP = np.float16
USE_FP8_A = False  # fp8 for the phase-A (T) gather (1.6e-2 rel err; too risky)


def _cdiv(a, b):
    return (a + b - 1) // b


def _r16(a):
    return (a + 15) // 16 * 16


# ----------------------------------------------------------------------------
# Host-side preprocessing: build per-core token tables from vidx/eidx.
# ----------------------------------------------------------------------------
class Prep:
    pass


def host_prep(vidx, eidx, N, E, C):
    """Static segment/gather structure shared by the SPMD program.

    Phase A (v->e): per core, entries sorted by eidx, grouped into NBE blocks
    of 128 edges. Per block, the token count is r16A[b] = roundup16 of the max
    per-core count; per-core tables are padded to r16A with small real indices
    and row-position -1 (one-hot never matches, so padding contributes zero).
    Phase B (e->v) is the same with (node block, eidx) swapped.
    """
    p = Prep()
    NP = N // C
    NBE_real = _cdiv(E, P)          # 157 edge blocks hold real edges
    NBE = _cdiv(NBE_real, 16) * 16  # padded to 160 so AR chunks split evenly
    NBV = _cdiv(NP, P)
    NPAD = NBV * P
    EPAD = NBE * P                  # 20480
    p.N, p.E, p.C, p.NP = N, E, C, NP
    p.NBE_real, p.NBE, p.NBV, p.NPAD, p.EPAD = NBE_real, NBE, NBV, NPAD, EPAD

    vidx = np.asarray(vidx).astype(np.int64)
    eidx = np.asarray(eidx).astype(np.int64)
    de = np.bincount(eidx, minlength=E).astype(np.float64)
    dv = np.bincount(vidx, minlength=N).astype(np.float64)
    de_inv = (1.0 / np.maximum(de, 1.0)).astype(np.float32)
    dv_inv = (1.0 / np.maximum(dv, 1.0)).astype(np.float32)
    core = vidx // NP

    def build_tables(key_all, val_all, nblocks):
        # key: block id = key_all // P decides the block; val: gather index
        # returns r16 (per-block padded counts), slot counts, offsets, and
        # per-core idx table + one-hot row-position table
        cnt = np.zeros((C, nblocks), np.int64)
        keys, vals = [], []
        for c in range(C):
            k = key_all[c]
            o = np.argsort(k, kind="stable")
            k = k[o]
            v = val_all[c][o]
            cnt[c] = np.bincount(k // P, minlength=nblocks)
            keys.append(k)
            vals.append(v)
        r16 = np.array([_r16(max(int(cnt[:, b].max()), 16)) for b in range(nblocks)])
        slots = (r16 + P - 1) // P
        tabOff = np.zeros(nblocks + 1, np.int64)
        np.cumsum(r16, out=tabOff[1:])
        slotOff = np.zeros(nblocks + 1, np.int64)
        np.cumsum(slots, out=slotOff[1:])
        T16 = int(tabOff[-1])
        SL = int(slotOff[-1])
        # padding gathers real rows 0..127 (spread across banks); the one-hot
        # rpos stays -1 there so padding contributes zero. (-1 indices are
        # skipped by the DMA but wedge the device when used across many calls)
        idx = np.tile((np.arange(T16) % P).astype(np.int16), (C, 1))
        rpos = np.full((C, SL * P), -1.0, np.float32)
        for c in range(C):
            k, v = keys[c], vals[c]
            blk = k // P
            starts = np.searchsorted(k, np.arange(nblocks) * P)
            within = np.arange(len(k)) - starts[blk]
            idx[c, tabOff[blk] + within] = v
            rpos[c, slotOff[blk] * P + within] = k - blk * P
        return r16, slots, tabOff, slotOff, T16, SL, idx, rpos

    # ---- phase A: tokens keyed by edge, gather local node rows of T ----
    keyA = [eidx[core == c] for c in range(C)]
    valA = [(vidx[core == c] - c * NP) for c in range(C)]
    (p.r16A, p.slotsA, p.tabOffA, p.slotOffA, p.TA16, p.SLA,
     idxA, rposA) = build_tables(keyA, valA, NBE_real)

    # ---- phase B: tokens keyed by local node, gather edge rows of YeF ----
    keyB = [(vidx[core == c] - c * NP) for c in range(C)]
    valB = [eidx[core == c] for c in range(C)]
    (p.r16B, p.slotsB, p.tabOffB, p.slotOffB, p.TB16, p.SLB,
     idxB, rposB) = build_tables(keyB, valB, NBV)

    p.MAXSLOT = int(max(p.slotsA.max(), p.slotsB.max()))

    # device layouts: idx wrapped into 16 partitions (replicated to 128);
    # rpos as [128, slots] columns
    def wrap_idx(idx, T16):
        return np.ascontiguousarray(
            np.tile(idx.reshape(C, T16 // 16, 16).transpose(0, 2, 1), (1, 8, 1)))

    p.idxA_w = wrap_idx(idxA, p.TA16)
    p.idxB_w = wrap_idx(idxB, p.TB16)
    p.rA_m = np.ascontiguousarray(
        rposA.reshape(C, p.SLA, P).transpose(0, 2, 1)).astype(F16_NP)
    p.rB_m = np.ascontiguousarray(
        rposB.reshape(C, p.SLB, P).transpose(0, 2, 1)).astype(F16_NP)

    # de_inv per edge-block column [128, NBE]; dv_inv per node [C, 128, NBV]
    dec = np.zeros(EPAD, np.float32)
    dec[:E] = de_inv
    p.dec = dec.reshape(NBE, P).T.copy()
    dvc = np.zeros((C, P, NBV), np.float32)
    for c in range(C):
        ids = c * NP + np.arange(NPAD)
        vals = np.where(ids < (c + 1) * NP, dv_inv[np.minimum(ids, N - 1)], 0.0)
        dvc[c] = vals.reshape(NBV, P).T
    p.dvc = dvc
    return p


# ----------------------------------------------------------------------------
# Device program
# ----------------------------------------------------------------------------
def build_program(p, IN_DIM, H, OUT, L, stage=99):
    C, NP, NBV, NPAD, EPAD = p.C, p.NP, p.NBV, p.NPAD, p.EPAD
    NBE_real = p.NBE_real
    KI = IN_DIM // P  # 3
    KH = H // P       # 2
    GDTA = F8 if USE_FP8_A else F16
    ECHUNK = EPAD // 2  # AllReduce chunk (rows)
    EBLK_CH = ECHUNK // P  # 80 edge blocks per chunk

    nc = bacc.Bacc(
        "TRN2",
        target_bir_lowering=False,
        debug=False,
        enable_asserts=False,
        num_devices=C,
        num_swdge_queues=4,
    )

    # ---- I/O ----
    xT_d = nc.dram_tensor("xT", [IN_DIM, NPAD], F16, kind="ExternalInput")
    encW_d = nc.dram_tensor("encW", [IN_DIM, H], F16, kind="ExternalInput")
    encB_d = nc.dram_tensor("encB", [H], F32, kind="ExternalInput")
    thW_d = nc.dram_tensor("thW", [L, H, H], F16, kind="ExternalInput")
    thB_d = nc.dram_tensor("thB", [L, H], F16, kind="ExternalInput")
    # LN affine pre-transposed on host: [P, L*KH], column (i*KH + m) holds
    # features m*128..(m+1)*128 of layer i
    lnG_d = nc.dram_tensor("lnGT", [P, L * KH], F32, kind="ExternalInput")
    lnB_d = nc.dram_tensor("lnBT", [P, L * KH], F32, kind="ExternalInput")
    linW_d = nc.dram_tensor("linW", [H, OUT], F16, kind="ExternalInput")
    linB_d = nc.dram_tensor("linB", [OUT], F16, kind="ExternalInput")
    idxA_d = nc.dram_tensor("idxA", [P, p.TA16 // 16], I16, kind="ExternalInput")
    rA_d = nc.dram_tensor("rA", [P, p.SLA], F16, kind="ExternalInput")
    idxB_d = nc.dram_tensor("idxB", [P, p.TB16 // 16], I16, kind="ExternalInput")
    rB_d = nc.dram_tensor("rB", [P, p.SLB], F16, kind="ExternalInput")
    dv_d = nc.dram_tensor("dvc", [P, NBV], F32, kind="ExternalInput")
    dec_d = nc.dram_tensor("dec", [P, p.NBE], F32, kind="ExternalInput")
    out_d = nc.dram_tensor("out", [NP, OUT], F32, kind="ExternalOutput")

    # ---- internals ----
    T_d = nc.dram_tensor("T_t", [NPAD, H], GDTA)
    YePa_d = nc.dram_tensor("YePa", [ECHUNK, H], F16)
    YePb_d = nc.dram_tensor("YePb", [ECHUNK, H], F16)
    YeF_d = nc.dram_tensor("YeF", [EPAD, H], F16, addr_space="Shared")
    h_d = nc.dram_tensor("h_t", [NPAD, H], F16)

    last_rows = NP - (NBV - 1) * P

    from contextlib import ExitStack
    with tile.TileContext(nc) as tc, ExitStack() as es:
        const = es.enter_context(tc.tile_pool(name="const", bufs=1))
        meta = es.enter_context(tc.tile_pool(name="meta", bufs=1))
        gpa = es.enter_context(tc.tile_pool(name="gpa", bufs=4))
        gpb = es.enter_context(tc.tile_pool(name="gpb", bufs=4))
        spool = es.enter_context(tc.tile_pool(name="spool", bufs=3))
        wrk = es.enter_context(tc.tile_pool(name="wrk", bufs=3))
        stat = es.enter_context(tc.tile_pool(name="stat", bufs=4))
        opool = es.enter_context(tc.tile_pool(name="opool", bufs=3))
        psA = es.enter_context(tc.tile_pool(name="psA", bufs=3, space="PSUM"))
        psT = es.enter_context(tc.tile_pool(name="psT", bufs=2, space="PSUM"))
        psE = es.enter_context(tc.tile_pool(name="psE", bufs=2, space="PSUM"))

        MS = p.MAXSLOT
        # ---- constants ----
        iota_i = const.tile([P, MS, P], I32)
        nc.gpsimd.iota(iota_i[:, :, :], pattern=[[0, MS], [1, P]], base=0,
                       channel_multiplier=0)
        iota_f = const.tile([P, MS, P], F16)
        nc.vector.tensor_copy(iota_f[:, :, :], iota_i[:, :, :])
        ident = const.tile([P, P], F16)
        make_identity(nc, ident[:, :])
        ones1 = const.tile([1, P], F16)
        nc.vector.memset(ones1[:, :], 1.0)
        epsc = const.tile([P, 1], F32)
        nc.vector.memset(epsc[:, :], 1e-5)

        # persistent transposed activations hT [feat_chunk, node]
        hT_sb = const.tile([P, KH, NPAD], F16)

        # zero G pools once so padding rows are finite (0 * garbage != NaN)
        for b in range(4):
            ga = gpa.tile([P, p.slotsA.max(), H], GDTA, tag="GA")
            nc.vector.memset(ga[:, :, :], 0.0)
            gb = gpb.tile([P, p.slotsB.max(), H], F16, tag="GB")
            nc.vector.memset(gb[:, :, :], 0.0)

        # weights
        encW_t = []
        for k in range(KI):
            row = []
            for m in range(KH):
                t = const.tile([P, P], F16, tag=f"encW{k}{m}")
                nc.sync.dma_start(t[:, :], encW_d[k * P:(k + 1) * P, m * P:(m + 1) * P])
                row.append(t)
            encW_t.append(row)
        encB_c = []
        for m in range(KH):
            t = const.tile([P, 1], F32, tag=f"encB{m}")
            nc.sync.dma_start(t[:, :], encB_d[m * P:(m + 1) * P, None])
            encB_c.append(t)
        thW_t = []
        for i in range(L):
            row = []
            for k in range(KH):
                t = const.tile([P, H], F16, tag=f"thW{i}{k}")
                nc.sync.dma_start(t[:, :], thW_d[i, k * P:(k + 1) * P, :])
                row.append(t)
            thW_t.append(row)
        thB_t = []
        for i in range(L):
            t = const.tile([1, H], F16, tag=f"thB{i}")
            nc.sync.dma_start(t[:, :], thB_d[i:i + 1, :])
            thB_t.append(t)
        linW_t = []
        for k in range(KH):
            t = const.tile([P, OUT], F16, tag=f"linW{k}")
            nc.sync.dma_start(t[:, :], linW_d[k * P:(k + 1) * P, :])
            linW_t.append(t)
        linB_t = const.tile([1, OUT], F16)
        nc.sync.dma_start(linB_t[:, :], linB_d[None, :])
        # LN affine in transposed space: per-feature -> per-partition columns
        lnG_t, lnB_t = [], []
        for i in range(L):
            g = const.tile([P, KH], F32, tag=f"lnG{i}")
            b = const.tile([P, KH], F32, tag=f"lnB{i}")
            nc.sync.dma_start(g[:, :], lnG_d[:, i * KH:(i + 1) * KH])
            nc.sync.dma_start(b[:, :], lnB_d[:, i * KH:(i + 1) * KH])
            lnG_t.append(g)
            lnB_t.append(b)

        # metadata
        idxA_t = meta.tile([P, p.TA16 // 16], I16)
        nc.sync.dma_start(idxA_t[:, :], idxA_d[:, :])
        rA_t = meta.tile([P, p.SLA], F16)
        nc.sync.dma_start(rA_t[:, :], rA_d[:, :])
        dec_t = meta.tile([P, p.NBE], F32)
        nc.sync.dma_start(dec_t[:, :], dec_d[:, :])
        idxB_t = meta.tile([P, p.TB16 // 16], I16)
        nc.sync.dma_start(idxB_t[:, :], idxB_d[:, :])
        rB_t = meta.tile([P, p.SLB], F16)
        nc.sync.dma_start(rB_t[:, :], rB_d[:, :])
        dv_t = meta.tile([P, NBV], F32)
        nc.sync.dma_start(dv_t[:, :], dv_d[:, :])

        # zero the pad edge blocks of YePb (rows E..EPAD land there)
        zblk = const.tile([P, H], F16)
        nc.vector.memset(zblk[:, :], 0.0)
        r0 = NBE_real * P - ECHUNK  # first pad row within chunk b
        while r0 < ECHUNK:
            nr = min(P, ECHUNK - r0)
            nc.sync.dma_start(YePb_d[r0:r0 + nr, :], zblk[:nr, :])
            r0 += nr

        # ------------------------------------------------------------------
        # Encoder: hT[:, m, :] = (x @ encW + encB)^T, feature-major directly.
        # ------------------------------------------------------------------
        CW = 512
        for c0 in range(0, NPAD, CW):
            ncols = min(CW, NPAD - c0)
            xc = wrk.tile([P, KI, CW], F16, tag="xc")
            nc.sync.dma_start(
                xc[:, :, :ncols],
                xT_d.ap().rearrange("(k q) n -> q k n", q=P)[:, :, c0:c0 + ncols],
            )
            for m in range(KH):
                ps = psE.tile([P, CW], F32, tag="psE")
                for k in range(KI):
                    nc.tensor.matmul(ps[:, :ncols], lhsT=encW_t[k][m][:, :],
                                     rhs=xc[:, k, :ncols],
                                     start=(k == 0), stop=(k == KI - 1))
                nc.scalar.activation(hT_sb[:, m, c0:c0 + ncols], ps[:, :ncols],
                                     AF.Identity, bias=encB_c[m][:, :], scale=1.0)

        # ------------------------------------------------------------------
        # Conv layers
        # ------------------------------------------------------------------
        qn = 0
        for li in range(L if stage >= 20 else 0):
            # ---- T = h @ thetaW[li] + thetaB[li] (row-major, fp8/fp16) ----
            for rb in range(NBV):
                rows = last_rows if rb == NBV - 1 else P
                ps = psA.tile([P, H], F32, tag="ps256")
                for k in range(KH):
                    nc.tensor.matmul(ps[:, :], lhsT=hT_sb[:, k, rb * P:rb * P + P],
                                     rhs=thW_t[li][k][:, :],
                                     start=(k == 0), stop=False)
                nc.tensor.matmul(ps[:, :], lhsT=ones1[:1, :], rhs=thB_t[li][:1, :],
                                 start=False, stop=True)
                Tb = opool.tile([P, H], GDTA, tag="Tout")
                nc.scalar.activation(Tb[:, :], ps[:, :], AF.Copy)
                nc.sync.dma_start(T_d[rb * P:rb * P + rows, :], Tb[:rows, :])

            # ---- Phase A: partial Ye, chunked AllReduce ----
            # sub-stages: 25=gather only, 27=+S build, 3=full phase A
            for eb in (range(NBE_real) if stage >= 25 else []):
                sb = int(p.slotsA[eb])
                s0 = int(p.slotOffA[eb])
                t0 = int(p.tabOffA[eb])
                r16 = int(p.r16A[eb])
                G = gpa.tile([P, p.slotsA.max(), H], GDTA, tag="GA")
                nc.gpsimd.dma_gather(
                    out_ap=G[:, :sb, :],
                    in_ap=T_d[:, :],
                    idxs_ap=idxA_t[:, t0 // 16:(t0 + r16) // 16],
                    num_idxs=r16,
                    num_idxs_reg=r16,
                    elem_size=H,
                    queue_num=qn,
                )
                qn = (qn + 1) % 4
                if stage < 27:
                    continue
                S = spool.tile([P, MS, P], GDTA, tag="SA")
                rb_ap = rA_t[:, s0:s0 + sb].unsqueeze(2).broadcast_to([P, sb, P])
                nc.vector.tensor_tensor(S[:, :sb, :], iota_f[:, :sb, :], rb_ap,
                                        op=ALU.is_equal)
                if stage < 29:
                    continue
                ps = psA.tile([P, H], F32, tag="ps256")
                for s in range(sb):
                    nc.tensor.matmul(ps[:, :], lhsT=S[:, s, :], rhs=G[:, s, :],
                                     start=(s == 0), stop=(s == sb - 1))
                yeb = opool.tile([P, H], F16, tag="yeg")
                nc.scalar.activation(yeb[:, :], ps[:, :], AF.Copy,
                                     scale=dec_t[:, eb:eb + 1])
                ye_d = YePa_d if eb < EBLK_CH else YePb_d
                er = eb * P - (0 if eb < EBLK_CH else ECHUNK)
                nc.scalar.dma_start(ye_d[er:er + P, :], yeb[:, :])

                if stage >= 40 and eb == EBLK_CH - 1:
                    nc.gpsimd.collective_compute(
                        "AllReduce", ALU.add,
                        replica_groups=[list(range(C))],
                        ins=[YePa_d.ap()[:, :]],
                        outs=[YeF_d.ap()[:ECHUNK, :]],
                    )
            if stage < 40:
                continue
            nc.gpsimd.collective_compute(
                "AllReduce", ALU.add,
                replica_groups=[list(range(C))],
                ins=[YePb_d.ap()[:, :]],
                outs=[YeF_d.ap()[ECHUNK:, :]],
            )

            # ---- Phase B: conv + residual + LN tail ----
            lnxt = li + 1 if li + 1 < L else 0
            for vb in (range(NBV) if stage >= 50 else []):
                rows = last_rows if vb == NBV - 1 else P
                sb = int(p.slotsB[vb])
                s0 = int(p.slotOffB[vb])
                t0 = int(p.tabOffB[vb])
                r16 = int(p.r16B[vb])
                G = gpb.tile([P, p.slotsB.max(), H], F16, tag="GB")
                # the gather ucode handles at most 1024 idxs (8 slots) per call
                g0 = 0
                while g0 < r16:
                    gn = min(1024, r16 - g0)
                    nc.gpsimd.dma_gather(
                        out_ap=G[:, g0 // P:g0 // P + _cdiv(gn, P), :],
                        in_ap=YeF_d[:, :],
                        idxs_ap=idxB_t[:, (t0 + g0) // 16:(t0 + g0 + gn) // 16],
                        num_idxs=gn,
                        num_idxs_reg=gn,
                        elem_size=H,
                        queue_num=qn,
                    )
                    qn = (qn + 1) % 4
                    g0 += gn
                S = spool.tile([P, MS, P], F16, tag="SB")
                rb_ap = rB_t[:, s0:s0 + sb].unsqueeze(2).broadcast_to([P, sb, P])
                nc.vector.tensor_tensor(S[:, :sb, :], iota_f[:, :sb, :], rb_ap,
                                        op=ALU.is_equal)
                ps = psA.tile([P, H], F32, tag="ps256")
                for s in range(sb):
                    nc.tensor.matmul(ps[:, :], lhsT=S[:, s, :], rhs=G[:, s, :],
                                     start=(s == 0), stop=(s == sb - 1))
                # relu(dv * x) (== dv * relu(x), dv >= 0)
                hn = wrk.tile([P, H], F16, tag="hn")
                nc.scalar.activation(hn[:, :], ps[:, :], AF.Relu,
                                     scale=dv_t[:, vb:vb + 1])
                if li > 0:
                    hp = wrk.tile([P, H], F16, tag="hp")
                    nc.scalar.dma_start(hp[:rows, :], h_d[vb * P:vb * P + rows, :])
                    nc.vector.tensor_add(hn[:rows, :], hn[:rows, :], hp[:rows, :])
                nc.scalar.dma_start(h_d[vb * P:vb * P + rows, :], hn[:rows, :])

                if stage < 60:
                    continue
                # tail: z = (hn - mu) * rinv; hT = relu(g * z^T + b)
                st6 = stat.tile([P, 6], F32, tag="st6")
                nc.vector.bn_stats(st6[:, :], hn[:, :])
                mv = stat.tile([P, 2], F32, tag="mv")
                nc.vector.bn_aggr(mv[:, :], st6[:, :])
                rstd = stat.tile([P, 1], F32, tag="rstd")
                nc.scalar.activation(rstd[:, :], mv[:, 1:2], AF.Sqrt,
                                     bias=epsc[:, :], scale=1.0)
                rinv = stat.tile([P, 1], F32, tag="rinv")
                nc.vector.reciprocal(rinv[:, :], rstd[:, :])
                nmr = stat.tile([P, 1], F32, tag="nmr")
                nc.vector.tensor_scalar(nmr[:, :], mv[:, 0:1], rinv[:, :], -1.0,
                                        op0=ALU.mult, op1=ALU.mult)
                z = wrk.tile([P, H], F16, tag="z")
                nc.scalar.activation(z[:, :], hn[:, :], AF.Identity,
                                     bias=nmr[:, :], scale=rinv[:, :])
                for m in range(KH):
                    pst = psT.tile([P, P], F16, tag="psT")
                    nc.tensor.transpose(pst[:, :], z[:, m * P:(m + 1) * P], ident[:, :])
                    nc.scalar.activation(
                        hT_sb[:, m, vb * P:vb * P + P], pst[:, :], AF.Relu,
                        bias=lnB_t[lnxt][:, m:m + 1], scale=lnG_t[lnxt][:, m:m + 1])

        # ------------------------------------------------------------------
        # Final: out = t @ linW + linB (t == relu(LN_0(h)) already in hT)
        # ------------------------------------------------------------------
        for rb in range(NBV):
            rows = last_rows if rb == NBV - 1 else P
            ps = psA.tile([P, H], F32, tag="ps256")
            for k in range(KH):
                nc.tensor.matmul(ps[:, :OUT], lhsT=hT_sb[:, k, rb * P:rb * P + P],
                                 rhs=linW_t[k][:, :], start=(k == 0), stop=False)
            nc.tensor.matmul(ps[:, :OUT], lhsT=ones1[:1, :], rhs=linB_t[:1, :],
                             start=False, stop=True)
            ob = opool.tile([P, OUT], F32, tag="finout")
            nc.scalar.activation(ob[:, :], ps[:, :OUT], AF.Copy)
            nc.sync.dma_start(out_d[rb * P:rb * P + rows, :], ob[:rows, :])

    nc.compile()
    return nc


# ----------------------------------------------------------------------------
# Full pipeline: prep + build + run
# ----------------------------------------------------------------------------
def run_full(x, vidx, eidx, encW, encB, thetaW, thetaB, lnG, lnB, linW, linB,
             N, E, C, trace=False, nc_cache=None, stage=99, **runkw):
    IN_DIM = x.shape[1]
    H = encW.shape[1]
    OUT = linW.shape[1]
    L = thetaW.shape[0]

    p = host_prep(np.asarray(vidx), np.asarray(eidx), N, E, C)
    nc = nc_cache if nc_cache is not None else build_program(p, IN_DIM, H, OUT, L, stage=stage)

    x = np.asarray(x, np.float32)
    NP, NPAD = p.NP, p.NPAD
    in_maps = []
    for c in range(C):
        xs = x[c * NP:(c + 1) * NP]
        xT = np.zeros((IN_DIM, NPAD), F16_NP)
        xT[:, :NP] = xs.T.astype(F16_NP)
        in_maps.append(dict(
            xT=xT,
            encW=np.asarray(encW, F16_NP),
            encB=np.asarray(encB, np.float32),
            thW=np.asarray(thetaW, F16_NP),
            thB=np.asarray(thetaB, F16_NP),
            lnGT=np.ascontiguousarray(
                np.asarray(lnG, np.float32).reshape(4, 2, P).transpose(2, 0, 1).reshape(P, 8)),
            lnBT=np.ascontiguousarray(
                np.asarray(lnB, np.float32).reshape(4, 2, P).transpose(2, 0, 1).reshape(P, 8)),
            linW=np.asarray(linW, F16_NP),
            linB=np.asarray(linB, F16_NP),
            idxA=p.idxA_w[c],
            rA=p.rA_m[c],
            idxB=p.idxB_w[c],
            rB=p.rB_m[c],
            dvc=p.dvc[c],
            dec=p.dec,
        ))

    res = run_bass_kernel_spmd(nc, in_maps, core_ids=list(range(C)), trace=trace, **runkw)
    out = np.concatenate([res.results[c]["out"] for c in range(C)], axis=0)
    return out, res, nc, p


# hardcoded problem configuration (nn_DeeperHNN_88295937671288)
_N, _E, _NNZ = 100000, 20000, 800000
_C = 8

_nc_cache = None


def kernel(x, vidx, eidx, encW, encB, thetaW, thetaB, lnG, lnB, linW, linB):
    global _nc_cache
    out, res, nc, p = run_full(
        x, vidx, eidx, encW, encB, thetaW, thetaB, lnG, lnB, linW, linB,
        N=_N, E=_E, C=_C, nc_cache=None,
    )
    _nc_cache = nc
    return out.astype(np.float32)
